# revision 27
# baseline (speedup 1.0000x reference)
"""
DenseEquivariantIrrep kernel for 8x Trainium2 NeuronCores.

Math: the reference computes, per batch row b:
    y[b, f, s] = sum_{c,t} x[b, c, t] * W[c, t, f, s] + bias[f]
where W folds the group-Fourier transform (fwd), the per-irrep block
matmul with the kernel, and the inverse transform (inv).  W depends only
on (kernel, irreps) which are tiny, so it is folded on the host; the
device does the batch-scaled work: a [4096, 1536] x [1536, 1536] GEMM
per core (8-way batch-parallel, no cross-core communication).

Device pipeline per 128-row b-tile:
    DMA x natural [128,1536] f32 -> ACT cast to fp16
    -> ONE xbar DMA-transpose instruction to x^T tiles [128(ct), 12, 128(b)]
    -> x^T is the STATIONARY matmul operand, W streams (m-outer/j-inner,
       one PSUM accumulation group at a time): the output lands natural
       ([b, fs] on partitions x free) in PSUM, so no output transposes
    -> DVE fused bias-add on the PSUM->SBUF copyback -> DMA out.

Measured on HW (in-kernel repeat-loop wall difference): ~330 us/iter
vs 556,773 ns staged baseline; the 4-tile grouping gave the best stable
reps=257 walls of the session. PE-bound: 1152 N=512 fp16 matmuls at a
measured 224 ns each (~258 us) is the floor for this formulation.
"""

import numpy as np

import concourse.bass as bass
import concourse.mybir as mybir
from concourse import bacc
from concourse.tile import TileContext
from concourse.bass_utils import run_bass_kernel_spmd

N_CORES = 8
B, C, F, NS = 32768, 32, 32, 48
CT = C * NS   # 1536 contraction size
FS = F * NS   # 1536 output features
BS = B // N_CORES  # 4096 rows per core
KT = CT // 128     # 12 K tiles
MC = FS // 512     # 3 output chunks of 512 (one PSUM bank each)

W_DT = mybir.dt.float16  # GEMM operand dtype (~5e-4 rounding)


def _host_fold(kernel, bias, irreps_d1, irreps_d2, irreps_d3):
    """Fold fwd/inv Fourier matrices and kernel into W[(c,t),(f,s)] + bias."""
    groups = [np.asarray(irreps_d1), np.asarray(irreps_d2), np.asarray(irreps_d3)]
    n = NS
    fwd = np.concatenate(
        [g.transpose(1, 0, 2, 3).reshape(n, -1) for g in groups], axis=1
    ).astype(np.float64)
    inv = np.concatenate(
        [g.transpose(1, 0, 2, 3).reshape(n, -1) * (g.shape[-1] / n) for g in groups],
        axis=1,
    ).T.astype(np.float64)
    kh = np.asarray(kernel).astype(np.float64) @ fwd  # [F, C, 48]
    W = np.zeros((C, NS, F, NS), np.float64)
    off = 0
    for g in groups:
        ni, d = g.shape[0], g.shape[-1]
        for _ in range(ni):
            fw_n = fwd[:, off : off + d * d].reshape(n, d, d)         # [t, p, r]
            kh_n = kh[:, :, off : off + d * d].reshape(F, C, d, d)    # [f, c, r, q]
            iv_n = inv[off : off + d * d, :].reshape(d, d, n)         # [p, q, s]
            W += np.einsum("tpr,fcrq,pqs->ctfs", fw_n, kh_n, iv_n, optimize=True)
            off += d * d
    Wflat = np.ascontiguousarray(W.reshape(CT, FS)).astype(np.float32)
    # bias along the free (fs) dim, broadcast over the 128 partitions
    bias_fs = np.repeat(np.asarray(bias).astype(np.float32), NS)  # [FS], f-major
    bias_bc = np.ascontiguousarray(np.broadcast_to(bias_fs[None, :], (128, FS)))
    return Wflat, bias_bc


def make_in_maps(xf, Wflat, bias_bc):
    """Per-core input dicts. xf: [B, CT] f32 contiguous."""
    w16 = Wflat.astype(np.float16)
    return [
        {
            "x": xf[i * BS : (i + 1) * BS],
            "w": w16,
            "bias_bc": bias_bc,
        }
        for i in range(N_CORES)
    ]


def build_kernel(nc: bass.Bass, bs: int = BS, reps: int = 1):
    """Emit the per-core kernel into `nc`. bs = batch rows for this build.

    reps > 1 wraps the whole pipeline in a hardware loop (for timing)."""
    assert bs % 512 == 0
    import contextlib

    x_d = nc.dram_tensor("x", [bs, CT], mybir.dt.float32, kind="ExternalInput")
    w_d = nc.dram_tensor("w", [CT, FS], W_DT, kind="ExternalInput")
    b_d = nc.dram_tensor("bias_bc", [128, FS], mybir.dt.float32, kind="ExternalInput")
    y_d = nc.dram_tensor("y", [bs, FS], mybir.dt.float32, kind="ExternalOutput")

    nbt = bs // 128

    with TileContext(nc) as tc:
        with (
            tc.tile_pool(name="singles", bufs=1) as singles,
            tc.tile_pool(name="x32", bufs=2) as x32_pool,
            tc.tile_pool(name="x16", bufs=2) as x16_pool,
            tc.tile_pool(name="xt", bufs=2) as xt_pool,
            tc.tile_pool(name="ysb", bufs=2) as ysb_pool,
            tc.tile_pool(name="py", bufs=8, space="PSUM") as py_pool,
        ):
            w_sb = singles.tile([128, KT, FS], W_DT)
            for j in range(KT):
                nc.sync.dma_start(
                    out=w_sb[:, j, :], in_=w_d[j * 128 : (j + 1) * 128, :]
                )
            bias_sb = singles.tile([128, FS], mybir.dt.float32)
            nc.sync.dma_start(out=bias_sb, in_=b_d[:, :])

            rep_ctx = (
                tc.For_i(0, reps, 1, hint_engines=(mybir.EngineType.PE,))
                if reps > 1
                else contextlib.nullcontext()
            )
            with rep_ctx:
                # process b-tiles in groups of G: one DMA in, one xbar
                # transpose, one DMA out per G*128 rows. HWDGE descriptor
                # generation is a single shared unit (~0.6us/DMA instr +
                # ~0.9us sem prop), so fewer DMA instructions is better.
                G = 4
                for bp in range(nbt // G):
                    r0 = bp * 128 * G
                    x32 = x32_pool.tile([128, G, CT], mybir.dt.float32, tag="x32")
                    nc.sync.dma_start(
                        out=x32,
                        in_=x_d[r0 : r0 + 128 * G, :].rearrange("(k p) c -> p k c", k=G),
                    )
                    x16 = x16_pool.tile([128, G, CT], W_DT, tag="x16")
                    nc.scalar.copy(x16, x32)  # ACT cast f32 -> fp16
                    # one xbar DMA transpose for both tiles ([128, 3072]):
                    # out[a, k*KT+j, b] = x16[b, k, 128j + a]
                    xt = xt_pool.tile([128, G * KT, 128], W_DT, tag="xt")
                    nc.sync.dma_start(out=xt, in_=x16, transpose=True)
                    y_sb = ysb_pool.tile([128, G, FS], mybir.dt.float32, tag="ysb")
                    for k in range(G):
                        for m in range(MC):
                            py = py_pool.tile([128, 512], mybir.dt.float32, tag="py")
                            for j in range(KT):
                                nc.tensor.matmul(
                                    py,
                                    xt[:, k * KT + j, :],
                                    w_sb[:, j, m * 512 : (m + 1) * 512],
                                    start=(j == 0),
                                    stop=(j == KT - 1),
                                )
                            # fused bias add on the PSUM -> SBUF copyback (DVE)
                            nc.vector.scalar_tensor_tensor(
                                y_sb[:, k, m * 512 : (m + 1) * 512],
                                py,
                                1.0,
                                bias_sb[:, m * 512 : (m + 1) * 512],
                                mybir.AluOpType.mult,
                                mybir.AluOpType.add,
                            )
                    nc.sync.dma_start(
                        out=y_d[r0 : r0 + 128 * G, :].rearrange("(k p) c -> p k c", k=G),
                        in_=y_sb,
                    )
    return nc


def _run(x, Wflat, bias_bc, trace=False):
    nc = bacc.Bacc("TRN2", target_bir_lowering=False)
    build_kernel(nc, BS)
    nc.compile()
    xf = np.ascontiguousarray(x.reshape(B, CT))
    in_maps = make_in_maps(xf, Wflat, bias_bc)
    res = run_bass_kernel_spmd(nc, in_maps, list(range(N_CORES)), trace=trace)
    y = np.concatenate([res.results[i]["y"] for i in range(N_CORES)], axis=0)
    return y.reshape(B, F, NS), res


def kernel(x, kernel, bias, irreps_d1, irreps_d2, irreps_d3):
    Wflat, bias_bc = _host_fold(kernel, bias, irreps_d1, irreps_d2, irreps_d3)
    y, _ = _run(np.asarray(x, dtype=np.float32), Wflat, bias_bc)
    return y


# revision 28
# speedup vs baseline: 1.0200x; 1.0200x over previous
"""
DenseEquivariantIrrep kernel for 8x Trainium2 NeuronCores.

Math: the reference computes, per batch row b:
    y[b, f, s] = sum_{c,t} x[b, c, t] * W[c, t, f, s] + bias[f]
where W folds the group-Fourier transform (fwd), the per-irrep block
matmul with the kernel, and the inverse transform (inv).  W depends only
on (kernel, irreps) which are tiny, so it is folded on the host; the
device does the batch-scaled work: a [4096, 1536] x [1536, 1536] GEMM
per core (8-way batch-parallel, no cross-core communication).

Device pipeline per 128-row b-tile:
    DMA x natural [128,1536] f32 -> ACT cast to fp16
    -> ONE xbar DMA-transpose instruction to x^T tiles [128(ct), 12, 128(b)]
    -> x^T is the STATIONARY matmul operand, W streams (m-outer/j-inner,
       one PSUM accumulation group at a time): the output lands natural
       ([b, fs] on partitions x free) in PSUM, so no output transposes
    -> DVE fused bias-add on the PSUM->SBUF copyback -> DMA out.

Measured on HW (in-kernel repeat-loop wall difference): ~330 us/iter
vs 556,773 ns staged baseline; the 4-tile grouping gave the best stable
reps=257 walls of the session. PE-bound: 1152 N=512 fp16 matmuls at a
measured 224 ns each (~258 us) is the floor for this formulation.
"""

import numpy as np

import concourse.bass as bass
import concourse.mybir as mybir
from concourse import bacc
from concourse.tile import TileContext
from concourse.bass_utils import run_bass_kernel_spmd

N_CORES = 8
B, C, F, NS = 32768, 32, 32, 48
CT = C * NS   # 1536 contraction size
FS = F * NS   # 1536 output features
BS = B // N_CORES  # 4096 rows per core
KT = CT // 128     # 12 K tiles
MC = FS // 512     # 3 output chunks of 512 (one PSUM bank each)

W_DT = mybir.dt.float16  # GEMM operand dtype (~5e-4 rounding)


def _host_fold(kernel, bias, irreps_d1, irreps_d2, irreps_d3):
    """Fold fwd/inv Fourier matrices and kernel into W[(c,t),(f,s)] + bias."""
    groups = [np.asarray(irreps_d1), np.asarray(irreps_d2), np.asarray(irreps_d3)]
    n = NS
    fwd = np.concatenate(
        [g.transpose(1, 0, 2, 3).reshape(n, -1) for g in groups], axis=1
    ).astype(np.float64)
    inv = np.concatenate(
        [g.transpose(1, 0, 2, 3).reshape(n, -1) * (g.shape[-1] / n) for g in groups],
        axis=1,
    ).T.astype(np.float64)
    kh = np.asarray(kernel).astype(np.float64) @ fwd  # [F, C, 48]
    W = np.zeros((C, NS, F, NS), np.float64)
    off = 0
    for g in groups:
        ni, d = g.shape[0], g.shape[-1]
        for _ in range(ni):
            fw_n = fwd[:, off : off + d * d].reshape(n, d, d)         # [t, p, r]
            kh_n = kh[:, :, off : off + d * d].reshape(F, C, d, d)    # [f, c, r, q]
            iv_n = inv[off : off + d * d, :].reshape(d, d, n)         # [p, q, s]
            W += np.einsum("tpr,fcrq,pqs->ctfs", fw_n, kh_n, iv_n, optimize=True)
            off += d * d
    Wflat = np.ascontiguousarray(W.reshape(CT, FS)).astype(np.float32)
    # bias along the free (fs) dim, broadcast over the 128 partitions
    bias_fs = np.repeat(np.asarray(bias).astype(np.float32), NS)  # [FS], f-major
    bias_bc = np.ascontiguousarray(np.broadcast_to(bias_fs[None, :], (128, FS)))
    return Wflat, bias_bc


def make_in_maps(xf, Wflat, bias_bc):
    """Per-core input dicts. xf: [B, CT] f32 contiguous."""
    w16 = Wflat.astype(np.float16)
    return [
        {
            "x": xf[i * BS : (i + 1) * BS],
            "w": w16,
            "bias_bc": bias_bc,
        }
        for i in range(N_CORES)
    ]


def build_kernel(nc: bass.Bass, bs: int = BS, reps: int = 1):
    """Emit the per-core kernel into `nc`. bs = batch rows for this build.

    reps > 1 wraps the whole pipeline in a hardware loop (for timing)."""
    assert bs % 512 == 0
    import contextlib

    x_d = nc.dram_tensor("x", [bs, CT], mybir.dt.float32, kind="ExternalInput")
    w_d = nc.dram_tensor("w", [CT, FS], W_DT, kind="ExternalInput")
    b_d = nc.dram_tensor("bias_bc", [128, FS], mybir.dt.float32, kind="ExternalInput")
    y_d = nc.dram_tensor("y", [bs, FS], mybir.dt.float32, kind="ExternalOutput")

    nbt = bs // 128

    with TileContext(nc) as tc:
        with (
            tc.tile_pool(name="singles", bufs=1) as singles,
            tc.tile_pool(name="x16", bufs=3) as x16_pool,
            tc.tile_pool(name="xt", bufs=3) as xt_pool,
            tc.tile_pool(name="ysb", bufs=2) as ysb_pool,
            tc.tile_pool(name="py", bufs=8, space="PSUM") as py_pool,
        ):
            w_sb = singles.tile([128, KT, FS], W_DT)
            for j in range(KT):
                nc.sync.dma_start(
                    out=w_sb[:, j, :], in_=w_d[j * 128 : (j + 1) * 128, :]
                )
            bias_sb = singles.tile([128, FS], mybir.dt.float32)
            nc.sync.dma_start(out=bias_sb, in_=b_d[:, :])

            rep_ctx = (
                tc.For_i(0, reps, 1, hint_engines=(mybir.EngineType.PE,))
                if reps > 1
                else contextlib.nullcontext()
            )
            with rep_ctx:
                # process b-tiles in groups of G: one DMA in, one xbar
                # transpose, one DMA out per G*128 rows. HWDGE descriptor
                # generation is a single shared unit (~0.6us/DMA instr +
                # ~0.9us sem prop), so fewer DMA instructions is better.
                G = 4
                for bp in range(nbt // G):
                    r0 = bp * 128 * G
                    # gpsimd software-DGE DMA casts f32 -> fp16 in flight:
                    # no separate cast pass, no f32 staging buffer
                    x16 = x16_pool.tile([128, G, CT], W_DT, tag="x16")
                    nc.gpsimd.dma_start(
                        out=x16,
                        in_=x_d[r0 : r0 + 128 * G, :].rearrange("(k p) c -> p k c", k=G),
                    )
                    # one xbar DMA transpose for both tiles ([128, 3072]):
                    # out[a, k*KT+j, b] = x16[b, k, 128j + a]
                    xt = xt_pool.tile([128, G * KT, 128], W_DT, tag="xt")
                    nc.sync.dma_start(out=xt, in_=x16, transpose=True)
                    y_sb = ysb_pool.tile([128, G, FS], mybir.dt.float32, tag="ysb")
                    for k in range(G):
                        for m in range(MC):
                            py = py_pool.tile([128, 512], mybir.dt.float32, tag="py")
                            for j in range(KT):
                                nc.tensor.matmul(
                                    py,
                                    xt[:, k * KT + j, :],
                                    w_sb[:, j, m * 512 : (m + 1) * 512],
                                    start=(j == 0),
                                    stop=(j == KT - 1),
                                )
                            # fused bias add on the PSUM -> SBUF copyback (DVE)
                            nc.vector.scalar_tensor_tensor(
                                y_sb[:, k, m * 512 : (m + 1) * 512],
                                py,
                                1.0,
                                bias_sb[:, m * 512 : (m + 1) * 512],
                                mybir.AluOpType.mult,
                                mybir.AluOpType.add,
                            )
                    nc.sync.dma_start(
                        out=y_d[r0 : r0 + 128 * G, :].rearrange("(k p) c -> p k c", k=G),
                        in_=y_sb,
                    )
    return nc


def _run(x, Wflat, bias_bc, trace=False):
    nc = bacc.Bacc("TRN2", target_bir_lowering=False)
    build_kernel(nc, BS)
    nc.compile()
    xf = np.ascontiguousarray(x.reshape(B, CT))
    in_maps = make_in_maps(xf, Wflat, bias_bc)
    res = run_bass_kernel_spmd(nc, in_maps, list(range(N_CORES)), trace=trace)
    y = np.concatenate([res.results[i]["y"] for i in range(N_CORES)], axis=0)
    return y.reshape(B, F, NS), res


def kernel(x, kernel, bias, irreps_d1, irreps_d2, irreps_d3):
    Wflat, bias_bc = _host_fold(kernel, bias, irreps_d1, irreps_d2, irreps_d3)
    y, _ = _run(np.asarray(x, dtype=np.float32), Wflat, bias_bc)
    return y
